# revision 52
# baseline (speedup 1.0000x reference)
"""AttentionMIL pooling kernel for 8 TRN2 NeuronCores.

Reference math (per bag b of B=16, N=16384 instances, F=256 feats, L=128):
    A_V = tanh(x @ V); A_U = sigmoid(x @ U)      [N, L]
    s   = (A_V * A_U) @ w                        [N, 1]
    A   = softmax(s, axis=N);  Z = sum_n A[n] * x[n]
Returns (Z [B,F], A [B,N,1]).

Sharding: data-parallel over bags, 2 bags/core, weights replicated.

Per-core design (memory-bound: 32 MiB x-shard -> ~93us HBM floor):
  - SWDGE cast-DMA loads x f32->bf16 (Pool ring), 64-tile chunks; a ones
    column is appended to each tile so pooling also yields the softmax
    denominator for free
  - HWDGE xbar-transpose bf16 [128n,128f]->[128f,128n] per f-chunk (SP ring,
    decoupled from the load ring)
  - PE: A-layout matmul, lhsT = xT chunk (stationary), rhs = [V|U] chunk
    (moving, N'=256), K=256 via 2 accumulating chunks -> PSUM [n, 256]
  - ACT (single exp_and_others table set, no switches): tanh(zV),
    tanh(zU/2) (sigmoid via 0.5*(1+tanh(z/2))), and per-chunk Exp
  - DVE per 8-tile double-group: u1 = tU'+1; m1 = tV*(0.5w); m2 = m1*u1;
    half-fold add then reduce_X -> staged score columns
  - online softmax-pooling (no max subtraction; |s| <= ||w||_1 ~ 9):
    per 16-tile sub-chunk, exp(s)->bf16 columns; pooling matmuls lag 2
    sub-chunks: lhsT = exp column [128,1], rhs = x_nat|ones [128,257]
    accumulate U_f and D in one PSUM row; Z = U*(1/D), A = exp*(1/D)
"""

import sys

import numpy as np

try:  # concourse (bass/tile) must be importable; fall back to the repo path
    import concourse.bass  # noqa: F401
except ImportError:
    sys.path.insert(0, "/opt/trn_rl_repo")

B, N, F, L = 16, 16384, 256, 128
NCORES = 8
BAGS_PER_CORE = B // NCORES  # 2
P = 128                      # instances per tile (partition dim)
NT = N // P                  # 128 n-tiles per bag
GRP = 4                      # tiles per ACT/DVE batch group
NG = NT // GRP               # 32 groups per bag
FC = F // P                  # 2 f-chunks of 128
CH = 64                      # tiles per load chunk


def build_bass():
    import concourse.bass as bass
    import concourse.bacc as bacc
    import concourse.tile as tile
    from concourse import mybir
    from contextlib import ExitStack

    f32 = mybir.dt.float32
    bf16 = mybir.dt.bfloat16
    AF = mybir.ActivationFunctionType
    ALU = mybir.AluOpType

    nc = bacc.Bacc()
    x_in = nc.declare_dram_parameter("x", [BAGS_PER_CORE, N, F], f32, isOutput=False)
    V_in = nc.declare_dram_parameter("attention_V", [F, L], f32, isOutput=False)
    U_in = nc.declare_dram_parameter("attention_U", [F, L], f32, isOutput=False)
    w_in = nc.declare_dram_parameter("attention_w", [L, 1], f32, isOutput=False)
    Z_out = nc.declare_dram_parameter("Z", [BAGS_PER_CORE, F], f32, isOutput=True)
    A_out = nc.declare_dram_parameter("A", [BAGS_PER_CORE, N], f32, isOutput=True)

    with tile.TileContext(nc) as tc, ExitStack() as ctx:
        singles = ctx.enter_context(tc.tile_pool(name="singles", bufs=1))
        xnat_pool = ctx.enter_context(tc.tile_pool(name="xnat", bufs=2 * (128 // 64)))
        xt_pool = ctx.enter_context(tc.tile_pool(name="xt", bufs=64))
        act_pool = ctx.enter_context(tc.tile_pool(name="acts", bufs=4))
        stage_pool = ctx.enter_context(tc.tile_pool(name="stage", bufs=2))
        small_pool = ctx.enter_context(tc.tile_pool(name="small", bufs=2))
        out_pool = ctx.enter_context(tc.tile_pool(name="outs", bufs=2))
        psum_vu = ctx.enter_context(tc.tile_pool(name="psum_vu", bufs=3, space="PSUM"))
        psum_z = ctx.enter_context(tc.tile_pool(name="psum_z", bufs=1, space="PSUM"))
        psum_m = ctx.enter_context(tc.tile_pool(name="psum_m", bufs=1, space="PSUM"))

        # ---- one-time setup ----
        # Combined [V|U] weights, bf16, chunked over K=F: vu[:, c, 0:128]=V[c], [:,128:256]=U[c]
        vu_w = singles.tile([P, FC, 2 * L], bf16)
        for c in range(FC):
            nc.gpsimd.dma_start(out=vu_w[:, c, 0:L], in_=V_in[c * P:(c + 1) * P, :])
            nc.gpsimd.dma_start(out=vu_w[:, c, L:2 * L], in_=U_in[c * P:(c + 1) * P, :])
        for c in range(FC):
            # pre-scale U by 0.5 so tanh(zU/2) = tanh(x @ (U/2)) needs no
            # separate ACT scale -> V and U share one Tanh call per group
            nc.vector.tensor_scalar_mul(out=vu_w[:, c, L:2 * L],
                                        in0=vu_w[:, c, L:2 * L], scalar1=0.5)

        # w as a row [1, L], then broadcast to all partitions via K=1 matmul
        w_row = singles.tile([1, L], bf16)
        nc.gpsimd.dma_start(out=w_row, in_=w_in[:].rearrange("l o -> o l"))
        ones_row = singles.tile([1, P], bf16)
        nc.vector.memset(ones_row, 1.0)

        ps_w = psum_m.tile([P, L], f32, tag="misc")
        nc.tensor.matmul(ps_w, lhsT=ones_row, rhs=w_row, start=True, stop=True)
        # W512h: 0.5*w replicated on every partition, tiled GRP x along free
        # (the 0.5 comes from sigmoid(z) = 0.5*(1 + tanh(z/2)))
        w512x2 = singles.tile([P, 2, GRP, L], bf16)
        for h in range(2):
            for g in range(GRP):
                nc.vector.tensor_scalar_mul(out=w512x2[:, h, g, :], in0=ps_w, scalar1=0.5)

        NCH = NT // CH      # load chunks per bag
        PREF = 2            # chunks of load prefetch ahead of compute

        # Pre-allocate all x chunks (write-once) and set their ones-columns up
        # front so the load stream has no interleaved Pool-engine ops. Column F
        # of each tile is 1.0 so the pooling matmul also accumulates the
        # softmax denominator.
        xnat_all = [xnat_pool.tile([P, CH, F + 1], bf16, tag="xnat", name=f"xnat_{i}")
                    for i in range(BAGS_PER_CORE * NCH)]
        for xq in xnat_all:
            nc.vector.memset(xq[:, :, F:F + 1], 1.0)

        def issue_load(b, q):
            # SWDGE cast-DMA: f32 DRAM -> bf16 SBUF, on the Pool ring so the
            # SP HWDGE ring carries only transposes.
            xq = xnat_all[b * NCH + q]
            n0 = q * CH * P
            nc.gpsimd.dma_start(
                out=xq[:, :, 0:F],
                in_=x_in[b, n0:n0 + CH * P, :].rearrange("(t p) f -> p t f", p=P),
            )
            return xq

        for b in range(BAGS_PER_CORE):
            xnat = [issue_load(b, q) for q in range(PREF)]
            s_stage = stage_pool.tile([P, NT], f32, tag="s_stage")

            e_bf = stage_pool.tile([P, NT], bf16, tag="e_bf")
            ps_zacc = psum_z.tile([1, F + 1], f32, tag="ps_zacc")
            SUB = 16          # tiles per exp/pooling sub-chunk
            LAG = 2           # sub-chunks between exp and pooling matmuls
            NSUB = NT // SUB

            def pool_sub(s):
                # online pooling: U_f (+ denominator in column F) accumulate
                # across all 128 tiles of the bag in one PSUM row
                for ts_ in range(SUB):
                    t = s * SUB + ts_
                    xq = xnat[t // CH]
                    nc.tensor.matmul(
                        ps_zacc,
                        lhsT=e_bf[:, t:t + 1],
                        rhs=xq[:, t % CH, :],
                        start=(t == 0),
                        stop=(t == NT - 1),
                    )

            # ---- phase A + online pooling ----
            for q in range(NCH):
                if q + PREF < NCH:
                    xnat.append(issue_load(b, q + PREF))
                xq = xnat[q]
                for g2 in range(q * CH // (2 * GRP), (q + 1) * CH // (2 * GRP)):
                    # exp + lagged pooling at sub-chunk granularity
                    if (g2 * 2 * GRP) % SUB == 0:
                        s = g2 * 2 * GRP // SUB
                        if s >= 1:
                            nc.scalar.activation(
                                out=e_bf[:, (s - 1) * SUB:s * SUB],
                                in_=s_stage[:, (s - 1) * SUB:s * SUB],
                                func=AF.Exp,
                            )
                        if s >= LAG + 1:
                            pool_sub(s - LAG - 1)
                    # DVE elementwise/reduce ops batch over a PAIR of psum
                    # groups to amortize per-op overhead
                    tvu = act_pool.tile([P, 2, GRP, 2 * L], bf16, tag="tvu")
                    scratch = act_pool.tile([P, 2, GRP, L], bf16, tag="scratch")
                    m1 = act_pool.tile([P, 2, GRP, L], bf16, tag="m1")
                    for h in range(2):
                        g = 2 * g2 + h
                        ps = psum_vu.tile([P, GRP, 2 * L], f32, tag="ps_vu")
                        for i in range(GRP):
                            t = g * GRP + i
                            tq = t % CH
                            # transpose both f-chunks: [128n,128f]->[128f,128n]
                            xt = xt_pool.tile([P, FC, P], bf16, tag="xt")
                            for c in range(FC):
                                nc.sync.dma_start(
                                    out=xt[:, c, :],
                                    in_=xq[:, tq, c * P:(c + 1) * P],
                                    transpose=True,
                                )
                            # A-layout: out[n, 256] += xT_c.T @ [V|U]_c
                            for c in range(FC):
                                nc.tensor.matmul(
                                    ps[:, i, :],
                                    lhsT=xt[:, c, :],
                                    rhs=vu_w[:, c, :],
                                    start=(c == 0),
                                    stop=(c == FC - 1),
                                )
                        # one batched tanh over the whole [V|U] group;
                        # sigmoid(z) = 0.5*(1 + tanh(z/2)) keeps one table set
                        nc.scalar.activation(out=tvu[:, h], in_=ps, func=AF.Tanh)
                    tanh_v = tvu[:, :, :, 0:L]
                    tanh_u = tvu[:, :, :, L:2 * L]
                    nc.vector.tensor_scalar_add(out=scratch, in0=tanh_u, scalar1=1.0)
                    nc.vector.tensor_mul(out=m1, in0=tanh_v, in1=w512x2)
                    nc.vector.tensor_mul(out=scratch, in0=m1, in1=scratch)
                    # fold halves first: TT add runs 2x mode vs reduce's 1x
                    fold = act_pool.tile([P, 2, GRP, L // 2], bf16, tag="fold")
                    nc.vector.tensor_add(out=fold, in0=scratch[:, :, :, 0:L // 2],
                                         in1=scratch[:, :, :, L // 2:L])
                    nc.vector.tensor_reduce(
                        out=s_stage[:, g2 * 2 * GRP:(g2 + 1) * 2 * GRP],
                        in_=fold.rearrange("p a g l -> p (a g) l"),
                        axis=mybir.AxisListType.X,
                        op=ALU.add,
                    )
            nc.scalar.activation(
                out=e_bf[:, (NSUB - 1) * SUB:],
                in_=s_stage[:, (NSUB - 1) * SUB:],
                func=AF.Exp,
            )
            for s in range(NSUB - LAG - 1, NSUB):
                pool_sub(s)

            # ---- epilogue: Z = U/D, A = exp/D ----
            recip = small_pool.tile([1, 1], f32, tag="recip")
            nc.vector.reciprocal(out=recip, in_=ps_zacc[0:1, F:F + 1])
            z_sb = out_pool.tile([1, F], f32, tag="z_sb")
            nc.vector.tensor_scalar_mul(out=z_sb, in0=ps_zacc[0:1, 0:F], scalar1=recip)
            nc.sync.dma_start(out=Z_out[b:b + 1, :], in_=z_sb)

            recip_bc_ps = psum_m.tile([P, 1], f32, tag="misc")
            recip_bf = small_pool.tile([1, 1], bf16, tag="recip_bf")
            nc.vector.tensor_copy(out=recip_bf, in_=recip)
            nc.tensor.matmul(recip_bc_ps, lhsT=ones_row, rhs=recip_bf, start=True, stop=True)
            recip_bc = small_pool.tile([P, 1], f32, tag="recip_bc")
            nc.vector.tensor_copy(out=recip_bc, in_=recip_bc_ps)
            a_sc = out_pool.tile([P, NT], f32, tag="a_sc")
            nc.vector.tensor_scalar_mul(out=a_sc, in0=e_bf, scalar1=recip_bc)
            # a_sc[p, t] -> A[b, t*128 + p]
            nc.sync.dma_start(
                out=A_out[b, :].rearrange("(t p) -> p t", p=P),
                in_=a_sc,
            )
    nc.finalize()
    return nc


_CACHE = {}


def kernel(x, attention_V, attention_U, attention_w):
    from concourse.bass_utils import run_bass_kernel_spmd

    if "nc" not in _CACHE:
        _CACHE["nc"] = build_bass()
    nc = _CACHE["nc"]

    x = np.ascontiguousarray(np.asarray(x, dtype=np.float32))
    V = np.ascontiguousarray(np.asarray(attention_V, dtype=np.float32))
    U = np.ascontiguousarray(np.asarray(attention_U, dtype=np.float32))
    w = np.ascontiguousarray(np.asarray(attention_w, dtype=np.float32))

    in_maps = []
    for c in range(NCORES):
        in_maps.append({
            "x": x[c * BAGS_PER_CORE:(c + 1) * BAGS_PER_CORE],
            "attention_V": V,
            "attention_U": U,
            "attention_w": w,
        })
    res = run_bass_kernel_spmd(nc, in_maps, core_ids=list(range(NCORES)))
    Z = np.concatenate([r["Z"] for r in res.results], axis=0).astype(np.float32)
    A = np.concatenate([r["A"] for r in res.results], axis=0).astype(np.float32)
    return Z.reshape(B, F), A.reshape(B, N, 1)



# revision 53
# speedup vs baseline: 1.0025x; 1.0025x over previous
"""AttentionMIL pooling kernel for 8 TRN2 NeuronCores.

Reference math (per bag b of B=16, N=16384 instances, F=256 feats, L=128):
    A_V = tanh(x @ V); A_U = sigmoid(x @ U)      [N, L]
    s   = (A_V * A_U) @ w                        [N, 1]
    A   = softmax(s, axis=N);  Z = sum_n A[n] * x[n]
Returns (Z [B,F], A [B,N,1]).

Sharding: data-parallel over bags, 2 bags/core, weights replicated.

Per-core design (memory-bound: 32 MiB x-shard -> ~93us HBM floor):
  - SWDGE cast-DMA loads x f32->bf16 (Pool ring), 64-tile chunks; a ones
    column is appended to each tile so pooling also yields the softmax
    denominator for free
  - HWDGE xbar-transpose bf16 [128n,128f]->[128f,128n] per f-chunk (SP ring,
    decoupled from the load ring)
  - PE: A-layout matmul, lhsT = xT chunk (stationary), rhs = [V|U] chunk
    (moving, N'=256), K=256 via 2 accumulating chunks -> PSUM [n, 256]
  - ACT (single exp_and_others table set, no switches): tanh(zV),
    tanh(zU/2) (sigmoid via 0.5*(1+tanh(z/2))), and per-chunk Exp
  - DVE per 8-tile double-group: u1 = tU'+1; m1 = tV*(0.5w); m2 = m1*u1;
    half-fold add then reduce_X -> staged score columns
  - online softmax-pooling (no max subtraction; |s| <= ||w||_1 ~ 9):
    per 16-tile sub-chunk, exp(s)->bf16 columns; pooling matmuls lag 2
    sub-chunks: lhsT = exp column [128,1], rhs = x_nat|ones [128,257]
    accumulate U_f and D in one PSUM row; Z = U*(1/D), A = exp*(1/D)
"""

import sys

import numpy as np

try:  # concourse (bass/tile) must be importable; fall back to the repo path
    import concourse.bass  # noqa: F401
except ImportError:
    sys.path.insert(0, "/opt/trn_rl_repo")

B, N, F, L = 16, 16384, 256, 128
NCORES = 8
BAGS_PER_CORE = B // NCORES  # 2
P = 128                      # instances per tile (partition dim)
NT = N // P                  # 128 n-tiles per bag
GRP = 4                      # tiles per ACT/DVE batch group
NG = NT // GRP               # 32 groups per bag
FC = F // P                  # 2 f-chunks of 128
CH = 64                      # tiles per load chunk


def build_bass():
    import concourse.bass as bass
    import concourse.bacc as bacc
    import concourse.tile as tile
    from concourse import mybir
    from contextlib import ExitStack

    f32 = mybir.dt.float32
    bf16 = mybir.dt.bfloat16
    AF = mybir.ActivationFunctionType
    ALU = mybir.AluOpType

    nc = bacc.Bacc()
    x_in = nc.declare_dram_parameter("x", [BAGS_PER_CORE, N, F], f32, isOutput=False)
    V_in = nc.declare_dram_parameter("attention_V", [F, L], f32, isOutput=False)
    U_in = nc.declare_dram_parameter("attention_U", [F, L], f32, isOutput=False)
    w_in = nc.declare_dram_parameter("attention_w", [L, 1], f32, isOutput=False)
    Z_out = nc.declare_dram_parameter("Z", [BAGS_PER_CORE, F], f32, isOutput=True)
    A_out = nc.declare_dram_parameter("A", [BAGS_PER_CORE, N], f32, isOutput=True)

    with tile.TileContext(nc) as tc, ExitStack() as ctx:
        singles = ctx.enter_context(tc.tile_pool(name="singles", bufs=1))
        xnat_pool = ctx.enter_context(tc.tile_pool(name="xnat", bufs=2 * (128 // 64)))
        xt_pool = ctx.enter_context(tc.tile_pool(name="xt", bufs=64))
        act_pool = ctx.enter_context(tc.tile_pool(name="acts", bufs=4))
        stage_pool = ctx.enter_context(tc.tile_pool(name="stage", bufs=2))
        small_pool = ctx.enter_context(tc.tile_pool(name="small", bufs=2))
        out_pool = ctx.enter_context(tc.tile_pool(name="outs", bufs=2))
        psum_vu = ctx.enter_context(tc.tile_pool(name="psum_vu", bufs=3, space="PSUM"))
        psum_z = ctx.enter_context(tc.tile_pool(name="psum_z", bufs=1, space="PSUM"))
        psum_m = ctx.enter_context(tc.tile_pool(name="psum_m", bufs=1, space="PSUM"))

        # ---- one-time setup ----
        # Combined [V|U] weights, bf16, chunked over K=F: vu[:, c, 0:128]=V[c], [:,128:256]=U[c]
        vu_w = singles.tile([P, FC, 2 * L], bf16)
        for c in range(FC):
            nc.gpsimd.dma_start(out=vu_w[:, c, 0:L], in_=V_in[c * P:(c + 1) * P, :])
            nc.gpsimd.dma_start(out=vu_w[:, c, L:2 * L], in_=U_in[c * P:(c + 1) * P, :])
        for c in range(FC):
            # pre-scale U by 0.5 so tanh(zU/2) = tanh(x @ (U/2)) needs no
            # separate ACT scale -> V and U share one Tanh call per group
            nc.vector.tensor_scalar_mul(out=vu_w[:, c, L:2 * L],
                                        in0=vu_w[:, c, L:2 * L], scalar1=0.5)

        # w as a row [1, L], then broadcast to all partitions via K=1 matmul
        w_row = singles.tile([1, L], bf16)
        nc.gpsimd.dma_start(out=w_row, in_=w_in[:].rearrange("l o -> o l"))
        ones_row = singles.tile([1, P], bf16)
        nc.vector.memset(ones_row, 1.0)

        ps_w = psum_m.tile([P, L], f32, tag="misc")
        nc.tensor.matmul(ps_w, lhsT=ones_row, rhs=w_row, start=True, stop=True)
        # W512h: 0.5*w replicated on every partition, tiled GRP x along free
        # (the 0.5 comes from sigmoid(z) = 0.5*(1 + tanh(z/2)))
        w512x2 = singles.tile([P, 2, GRP, L], bf16)
        for h in range(2):
            for g in range(GRP):
                nc.vector.tensor_scalar_mul(out=w512x2[:, h, g, :], in0=ps_w, scalar1=0.5)

        NCH = NT // CH      # load chunks per bag
        PREF = 2            # chunks of load prefetch ahead of compute

        # Pre-allocate all x chunks (write-once) and set their ones-columns up
        # front so the load stream has no interleaved Pool-engine ops. Column F
        # of each tile is 1.0 so the pooling matmul also accumulates the
        # softmax denominator.
        xnat_all = [xnat_pool.tile([P, CH, F + 1], bf16, tag="xnat", name=f"xnat_{i}")
                    for i in range(BAGS_PER_CORE * NCH)]
        for xq in xnat_all:
            nc.vector.memset(xq[:, :, F:F + 1], 1.0)

        def issue_load(b, q):
            # SWDGE cast-DMA: f32 DRAM -> bf16 SBUF, on the Pool ring so the
            # SP HWDGE ring carries only transposes.
            xq = xnat_all[b * NCH + q]
            n0 = q * CH * P
            nc.gpsimd.dma_start(
                out=xq[:, :, 0:F],
                in_=x_in[b, n0:n0 + CH * P, :].rearrange("(t p) f -> p t f", p=P),
            )
            return xq

        for b in range(BAGS_PER_CORE):
            xnat = [issue_load(b, q) for q in range(PREF)]
            s_stage = stage_pool.tile([P, NT], f32, tag="s_stage")

            e_bf = stage_pool.tile([P, NT], bf16, tag="e_bf")
            ps_zacc = psum_z.tile([1, F + 1], f32, tag="ps_zacc")
            SUB = 16          # tiles per exp/pooling sub-chunk
            LAG = 2           # sub-chunks between exp and pooling matmuls
            NSUB = NT // SUB

            def pool_sub(s):
                # online pooling: U_f (+ denominator in column F) accumulate
                # across all 128 tiles of the bag in one PSUM row
                for ts_ in range(SUB):
                    t = s * SUB + ts_
                    xq = xnat[t // CH]
                    nc.tensor.matmul(
                        ps_zacc,
                        lhsT=e_bf[:, t:t + 1],
                        rhs=xq[:, t % CH, :],
                        start=(t == 0),
                        stop=(t == NT - 1),
                    )

            # ---- phase A + online pooling ----
            for q in range(NCH):
                if q + PREF < NCH:
                    xnat.append(issue_load(b, q + PREF))
                xq = xnat[q]
                for g2 in range(q * CH // (2 * GRP), (q + 1) * CH // (2 * GRP)):
                    # exp + lagged pooling at sub-chunk granularity
                    if (g2 * 2 * GRP) % SUB == 0:
                        s = g2 * 2 * GRP // SUB
                        if s >= 1:
                            nc.scalar.activation(
                                out=e_bf[:, (s - 1) * SUB:s * SUB],
                                in_=s_stage[:, (s - 1) * SUB:s * SUB],
                                func=AF.Exp,
                            )
                        if s >= LAG + 1:
                            pool_sub(s - LAG - 1)
                    # DVE elementwise/reduce ops batch over a PAIR of psum
                    # groups to amortize per-op overhead
                    tvu = act_pool.tile([P, 2, GRP, 2 * L], bf16, tag="tvu")
                    scratch = act_pool.tile([P, 2, GRP, L], bf16, tag="scratch")
                    m1 = act_pool.tile([P, 2, GRP, L], bf16, tag="m1")
                    for h in range(2):
                        g = 2 * g2 + h
                        ps = psum_vu.tile([P, GRP, 2 * L], f32, tag="ps_vu")
                        for i in range(GRP):
                            t = g * GRP + i
                            tq = t % CH
                            # transpose both f-chunks: [128n,128f]->[128f,128n]
                            xt = xt_pool.tile([P, FC, P], bf16, tag="xt")
                            for c in range(FC):
                                nc.sync.dma_start(
                                    out=xt[:, c, :],
                                    in_=xq[:, tq, c * P:(c + 1) * P],
                                    transpose=True,
                                )
                            # A-layout: out[n, 256] += xT_c.T @ [V|U]_c
                            for c in range(FC):
                                nc.tensor.matmul(
                                    ps[:, i, :],
                                    lhsT=xt[:, c, :],
                                    rhs=vu_w[:, c, :],
                                    start=(c == 0),
                                    stop=(c == FC - 1),
                                )
                        # one batched tanh over the whole [V|U] group;
                        # sigmoid(z) = 0.5*(1 + tanh(z/2)) keeps one table set
                        nc.scalar.activation(out=tvu[:, h], in_=ps, func=AF.Tanh)
                    tanh_v = tvu[:, :, :, 0:L]
                    tanh_u = tvu[:, :, :, L:2 * L]
                    nc.vector.tensor_scalar_add(out=scratch, in0=tanh_u, scalar1=1.0)
                    nc.vector.tensor_mul(out=m1, in0=tanh_v, in1=w512x2)
                    nc.vector.tensor_mul(out=scratch, in0=m1, in1=scratch)
                    # fold halves twice: TT adds run 2x mode vs reduce's 1x
                    fold = act_pool.tile([P, 2, GRP, L // 2], bf16, tag="fold")
                    nc.vector.tensor_add(out=fold, in0=scratch[:, :, :, 0:L // 2],
                                         in1=scratch[:, :, :, L // 2:L])
                    fold2 = act_pool.tile([P, 2, GRP, L // 4], bf16, tag="fold2")
                    nc.vector.tensor_add(out=fold2, in0=fold[:, :, :, 0:L // 4],
                                         in1=fold[:, :, :, L // 4:L // 2])
                    nc.vector.tensor_reduce(
                        out=s_stage[:, g2 * 2 * GRP:(g2 + 1) * 2 * GRP],
                        in_=fold2.rearrange("p a g l -> p (a g) l"),
                        axis=mybir.AxisListType.X,
                        op=ALU.add,
                    )
            nc.scalar.activation(
                out=e_bf[:, (NSUB - 1) * SUB:],
                in_=s_stage[:, (NSUB - 1) * SUB:],
                func=AF.Exp,
            )
            for s in range(NSUB - LAG - 1, NSUB):
                pool_sub(s)

            # ---- epilogue: Z = U/D, A = exp/D ----
            recip = small_pool.tile([1, 1], f32, tag="recip")
            nc.vector.reciprocal(out=recip, in_=ps_zacc[0:1, F:F + 1])
            z_sb = out_pool.tile([1, F], f32, tag="z_sb")
            nc.vector.tensor_scalar_mul(out=z_sb, in0=ps_zacc[0:1, 0:F], scalar1=recip)
            nc.sync.dma_start(out=Z_out[b:b + 1, :], in_=z_sb)

            recip_bc_ps = psum_m.tile([P, 1], f32, tag="misc")
            recip_bf = small_pool.tile([1, 1], bf16, tag="recip_bf")
            nc.vector.tensor_copy(out=recip_bf, in_=recip)
            nc.tensor.matmul(recip_bc_ps, lhsT=ones_row, rhs=recip_bf, start=True, stop=True)
            recip_bc = small_pool.tile([P, 1], f32, tag="recip_bc")
            nc.vector.tensor_copy(out=recip_bc, in_=recip_bc_ps)
            a_sc = out_pool.tile([P, NT], f32, tag="a_sc")
            nc.vector.tensor_scalar_mul(out=a_sc, in0=e_bf, scalar1=recip_bc)
            # a_sc[p, t] -> A[b, t*128 + p]
            nc.sync.dma_start(
                out=A_out[b, :].rearrange("(t p) -> p t", p=P),
                in_=a_sc,
            )
    nc.finalize()
    return nc


_CACHE = {}


def kernel(x, attention_V, attention_U, attention_w):
    from concourse.bass_utils import run_bass_kernel_spmd

    if "nc" not in _CACHE:
        _CACHE["nc"] = build_bass()
    nc = _CACHE["nc"]

    x = np.ascontiguousarray(np.asarray(x, dtype=np.float32))
    V = np.ascontiguousarray(np.asarray(attention_V, dtype=np.float32))
    U = np.ascontiguousarray(np.asarray(attention_U, dtype=np.float32))
    w = np.ascontiguousarray(np.asarray(attention_w, dtype=np.float32))

    in_maps = []
    for c in range(NCORES):
        in_maps.append({
            "x": x[c * BAGS_PER_CORE:(c + 1) * BAGS_PER_CORE],
            "attention_V": V,
            "attention_U": U,
            "attention_w": w,
        })
    res = run_bass_kernel_spmd(nc, in_maps, core_ids=list(range(NCORES)))
    Z = np.concatenate([r["Z"] for r in res.results], axis=0).astype(np.float32)
    A = np.concatenate([r["A"] for r in res.results], axis=0).astype(np.float32)
    return Z.reshape(B, F), A.reshape(B, N, 1)



# revision 57
# speedup vs baseline: 1.0076x; 1.0051x over previous
"""AttentionMIL pooling kernel for 8 TRN2 NeuronCores.

Reference math (per bag b of B=16, N=16384 instances, F=256 feats, L=128):
    A_V = tanh(x @ V); A_U = sigmoid(x @ U)      [N, L]
    s   = (A_V * A_U) @ w                        [N, 1]
    A   = softmax(s, axis=N);  Z = sum_n A[n] * x[n]
Returns (Z [B,F], A [B,N,1]).

Sharding: data-parallel over bags, 2 bags/core, weights replicated.

Per-core design (memory-bound: 32 MiB x-shard -> ~93us HBM floor):
  - SWDGE cast-DMA loads x f32->bf16 (Pool ring), 64-tile chunks; a ones
    column is appended to each tile so pooling also yields the softmax
    denominator for free
  - HWDGE xbar-transpose bf16 [128n,128f]->[128f,128n] per f-chunk (SP ring,
    decoupled from the load ring)
  - PE: A-layout matmul, lhsT = xT chunk (stationary), rhs = [V|U] chunk
    (moving, N'=256), K=256 via 2 accumulating chunks -> PSUM [n, 256]
  - ACT (single exp_and_others table set, no switches): tanh(zV),
    tanh(zU/2) (sigmoid via 0.5*(1+tanh(z/2))), and per-chunk Exp
  - DVE per 8-tile double-group: u1 = tU'+1; m1 = tV*(0.5w); m2 = m1*u1;
    two fold-adds (2x mode) then reduce_X -> staged score columns
  - online softmax-pooling (no max subtraction; |s| <= ||w||_1 ~ 9):
    per 8-tile sub-chunk, exp(s)->bf16 columns; pooling matmuls lag 2
    sub-chunks: lhsT = exp column [128,1], rhs = x_nat|ones [128,257]
    accumulate U_f and D in one PSUM row; Z = U*(1/D), A = exp*(1/D)
"""

import sys

import numpy as np

try:  # concourse (bass/tile) must be importable; fall back to the repo path
    import concourse.bass  # noqa: F401
except ImportError:
    sys.path.insert(0, "/opt/trn_rl_repo")

B, N, F, L = 16, 16384, 256, 128
NCORES = 8
BAGS_PER_CORE = B // NCORES  # 2
P = 128                      # instances per tile (partition dim)
NT = N // P                  # 128 n-tiles per bag
GRP = 4                      # tiles per ACT/DVE batch group
NG = NT // GRP               # 32 groups per bag
FC = F // P                  # 2 f-chunks of 128
CH = 64                      # tiles per load chunk


def build_bass():
    import concourse.bass as bass
    import concourse.bacc as bacc
    import concourse.tile as tile
    from concourse import mybir
    from contextlib import ExitStack

    f32 = mybir.dt.float32
    bf16 = mybir.dt.bfloat16
    AF = mybir.ActivationFunctionType
    ALU = mybir.AluOpType

    nc = bacc.Bacc()
    x_in = nc.declare_dram_parameter("x", [BAGS_PER_CORE, N, F], f32, isOutput=False)
    V_in = nc.declare_dram_parameter("attention_V", [F, L], f32, isOutput=False)
    U_in = nc.declare_dram_parameter("attention_U", [F, L], f32, isOutput=False)
    w_in = nc.declare_dram_parameter("attention_w", [L, 1], f32, isOutput=False)
    Z_out = nc.declare_dram_parameter("Z", [BAGS_PER_CORE, F], f32, isOutput=True)
    A_out = nc.declare_dram_parameter("A", [BAGS_PER_CORE, N], f32, isOutput=True)

    with tile.TileContext(nc) as tc, ExitStack() as ctx:
        singles = ctx.enter_context(tc.tile_pool(name="singles", bufs=1))
        xnat_pool = ctx.enter_context(tc.tile_pool(name="xnat", bufs=2 * (128 // 64)))
        xt_pool = ctx.enter_context(tc.tile_pool(name="xt", bufs=64))
        act_pool = ctx.enter_context(tc.tile_pool(name="acts", bufs=4))
        stage_pool = ctx.enter_context(tc.tile_pool(name="stage", bufs=2))
        small_pool = ctx.enter_context(tc.tile_pool(name="small", bufs=2))
        out_pool = ctx.enter_context(tc.tile_pool(name="outs", bufs=2))
        psum_vu = ctx.enter_context(tc.tile_pool(name="psum_vu", bufs=3, space="PSUM"))
        psum_z = ctx.enter_context(tc.tile_pool(name="psum_z", bufs=1, space="PSUM"))
        psum_m = ctx.enter_context(tc.tile_pool(name="psum_m", bufs=1, space="PSUM"))

        # ---- one-time setup ----
        # Combined [V|U] weights, bf16, chunked over K=F: vu[:, c, 0:128]=V[c], [:,128:256]=U[c]
        vu_w = singles.tile([P, FC, 2 * L], bf16)
        for c in range(FC):
            nc.gpsimd.dma_start(out=vu_w[:, c, 0:L], in_=V_in[c * P:(c + 1) * P, :])
            nc.gpsimd.dma_start(out=vu_w[:, c, L:2 * L], in_=U_in[c * P:(c + 1) * P, :])
        for c in range(FC):
            # pre-scale U by 0.5 so tanh(zU/2) = tanh(x @ (U/2)) needs no
            # separate ACT scale -> V and U share one Tanh call per group
            nc.vector.tensor_scalar_mul(out=vu_w[:, c, L:2 * L],
                                        in0=vu_w[:, c, L:2 * L], scalar1=0.5)

        # w as a row [1, L], then broadcast to all partitions via K=1 matmul
        w_row = singles.tile([1, L], bf16)
        nc.gpsimd.dma_start(out=w_row, in_=w_in[:].rearrange("l o -> o l"))
        ones_row = singles.tile([1, P], bf16)
        nc.vector.memset(ones_row, 1.0)

        ps_w = psum_m.tile([P, L], f32, tag="misc")
        nc.tensor.matmul(ps_w, lhsT=ones_row, rhs=w_row, start=True, stop=True)
        # W512h: 0.5*w replicated on every partition, tiled GRP x along free
        # (the 0.5 comes from sigmoid(z) = 0.5*(1 + tanh(z/2)))
        w512x2 = singles.tile([P, 2, GRP, L], bf16)
        for h in range(2):
            for g in range(GRP):
                nc.vector.tensor_scalar_mul(out=w512x2[:, h, g, :], in0=ps_w, scalar1=0.5)

        NCH = NT // CH      # load chunks per bag
        PREF = 2            # chunks of load prefetch ahead of compute

        # Pre-allocate all x chunks (write-once) and set their ones-columns up
        # front so the load stream has no interleaved Pool-engine ops. Column F
        # of each tile is 1.0 so the pooling matmul also accumulates the
        # softmax denominator.
        xnat_all = [xnat_pool.tile([P, CH, F + 1], bf16, tag="xnat", name=f"xnat_{i}")
                    for i in range(BAGS_PER_CORE * NCH)]
        for xq in xnat_all:
            nc.vector.memset(xq[:, :, F:F + 1], 1.0)

        def issue_load(b, q):
            # SWDGE cast-DMA: f32 DRAM -> bf16 SBUF, on the Pool ring so the
            # SP HWDGE ring carries only transposes.
            xq = xnat_all[b * NCH + q]
            n0 = q * CH * P
            nc.gpsimd.dma_start(
                out=xq[:, :, 0:F],
                in_=x_in[b, n0:n0 + CH * P, :].rearrange("(t p) f -> p t f", p=P),
            )
            return xq

        for b in range(BAGS_PER_CORE):
            xnat = [issue_load(b, q) for q in range(PREF)]
            s_stage = stage_pool.tile([P, NT], f32, tag="s_stage")

            e_bf = stage_pool.tile([P, NT], bf16, tag="e_bf")
            ps_zacc = psum_z.tile([1, F + 1], f32, tag="ps_zacc")
            SUB = 8           # tiles per exp/pooling sub-chunk
            LAG = 2           # sub-chunks between exp and pooling matmuls
            NSUB = NT // SUB

            def pool_sub(s):
                # online pooling: U_f (+ denominator in column F) accumulate
                # across all 128 tiles of the bag in one PSUM row
                for ts_ in range(SUB):
                    t = s * SUB + ts_
                    xq = xnat[t // CH]
                    nc.tensor.matmul(
                        ps_zacc,
                        lhsT=e_bf[:, t:t + 1],
                        rhs=xq[:, t % CH, :],
                        start=(t == 0),
                        stop=(t == NT - 1),
                    )

            # ---- phase A + online pooling ----
            for q in range(NCH):
                if q + PREF < NCH:
                    xnat.append(issue_load(b, q + PREF))
                xq = xnat[q]
                for g2 in range(q * CH // (2 * GRP), (q + 1) * CH // (2 * GRP)):
                    # exp + lagged pooling at sub-chunk granularity
                    if (g2 * 2 * GRP) % SUB == 0:
                        s = g2 * 2 * GRP // SUB
                        if s >= 1:
                            nc.scalar.activation(
                                out=e_bf[:, (s - 1) * SUB:s * SUB],
                                in_=s_stage[:, (s - 1) * SUB:s * SUB],
                                func=AF.Exp,
                            )
                        if s >= LAG + 1:
                            pool_sub(s - LAG - 1)
                    # DVE elementwise/reduce ops batch over a PAIR of psum
                    # groups to amortize per-op overhead
                    tvu = act_pool.tile([P, 2, GRP, 2 * L], bf16, tag="tvu")
                    scratch = act_pool.tile([P, 2, GRP, L], bf16, tag="scratch")
                    m1 = act_pool.tile([P, 2, GRP, L], bf16, tag="m1")
                    for h in range(2):
                        g = 2 * g2 + h
                        ps = psum_vu.tile([P, GRP, 2 * L], f32, tag="ps_vu")
                        for i in range(GRP):
                            t = g * GRP + i
                            tq = t % CH
                            # transpose both f-chunks: [128n,128f]->[128f,128n]
                            xt = xt_pool.tile([P, FC, P], bf16, tag="xt")
                            for c in range(FC):
                                nc.sync.dma_start(
                                    out=xt[:, c, :],
                                    in_=xq[:, tq, c * P:(c + 1) * P],
                                    transpose=True,
                                )
                            # A-layout: out[n, 256] += xT_c.T @ [V|U]_c
                            for c in range(FC):
                                nc.tensor.matmul(
                                    ps[:, i, :],
                                    lhsT=xt[:, c, :],
                                    rhs=vu_w[:, c, :],
                                    start=(c == 0),
                                    stop=(c == FC - 1),
                                )
                        # one batched tanh over the whole [V|U] group;
                        # sigmoid(z) = 0.5*(1 + tanh(z/2)) keeps one table set
                        nc.scalar.activation(out=tvu[:, h], in_=ps, func=AF.Tanh)
                    tanh_v = tvu[:, :, :, 0:L]
                    tanh_u = tvu[:, :, :, L:2 * L]
                    nc.vector.tensor_scalar_add(out=scratch, in0=tanh_u, scalar1=1.0)
                    nc.vector.tensor_mul(out=m1, in0=tanh_v, in1=w512x2)
                    nc.vector.tensor_mul(out=scratch, in0=m1, in1=scratch)
                    # fold halves twice: TT adds run 2x mode vs reduce's 1x
                    fold = act_pool.tile([P, 2, GRP, L // 2], bf16, tag="fold")
                    nc.vector.tensor_add(out=fold, in0=scratch[:, :, :, 0:L // 2],
                                         in1=scratch[:, :, :, L // 2:L])
                    fold2 = act_pool.tile([P, 2, GRP, L // 4], bf16, tag="fold2")
                    nc.vector.tensor_add(out=fold2, in0=fold[:, :, :, 0:L // 4],
                                         in1=fold[:, :, :, L // 4:L // 2])
                    nc.vector.tensor_reduce(
                        out=s_stage[:, g2 * 2 * GRP:(g2 + 1) * 2 * GRP],
                        in_=fold2.rearrange("p a g l -> p (a g) l"),
                        axis=mybir.AxisListType.X,
                        op=ALU.add,
                    )
            nc.scalar.activation(
                out=e_bf[:, (NSUB - 1) * SUB:],
                in_=s_stage[:, (NSUB - 1) * SUB:],
                func=AF.Exp,
            )
            for s in range(NSUB - LAG - 1, NSUB):
                pool_sub(s)

            # ---- epilogue: Z = U/D, A = exp/D ----
            recip = small_pool.tile([1, 1], f32, tag="recip")
            nc.vector.reciprocal(out=recip, in_=ps_zacc[0:1, F:F + 1])
            z_sb = out_pool.tile([1, F], f32, tag="z_sb")
            nc.vector.tensor_scalar_mul(out=z_sb, in0=ps_zacc[0:1, 0:F], scalar1=recip)
            nc.sync.dma_start(out=Z_out[b:b + 1, :], in_=z_sb)

            recip_bc_ps = psum_m.tile([P, 1], f32, tag="misc")
            recip_bf = small_pool.tile([1, 1], bf16, tag="recip_bf")
            nc.vector.tensor_copy(out=recip_bf, in_=recip)
            nc.tensor.matmul(recip_bc_ps, lhsT=ones_row, rhs=recip_bf, start=True, stop=True)
            recip_bc = small_pool.tile([P, 1], f32, tag="recip_bc")
            nc.vector.tensor_copy(out=recip_bc, in_=recip_bc_ps)
            a_sc = out_pool.tile([P, NT], f32, tag="a_sc")
            nc.vector.tensor_scalar_mul(out=a_sc, in0=e_bf, scalar1=recip_bc)
            # a_sc[p, t] -> A[b, t*128 + p]
            nc.sync.dma_start(
                out=A_out[b, :].rearrange("(t p) -> p t", p=P),
                in_=a_sc,
            )
    nc.finalize()
    return nc


_CACHE = {}


def kernel(x, attention_V, attention_U, attention_w):
    from concourse.bass_utils import run_bass_kernel_spmd

    if "nc" not in _CACHE:
        _CACHE["nc"] = build_bass()
    nc = _CACHE["nc"]

    x = np.ascontiguousarray(np.asarray(x, dtype=np.float32))
    V = np.ascontiguousarray(np.asarray(attention_V, dtype=np.float32))
    U = np.ascontiguousarray(np.asarray(attention_U, dtype=np.float32))
    w = np.ascontiguousarray(np.asarray(attention_w, dtype=np.float32))

    in_maps = []
    for c in range(NCORES):
        in_maps.append({
            "x": x[c * BAGS_PER_CORE:(c + 1) * BAGS_PER_CORE],
            "attention_V": V,
            "attention_U": U,
            "attention_w": w,
        })
    res = run_bass_kernel_spmd(nc, in_maps, core_ids=list(range(NCORES)))
    Z = np.concatenate([r["Z"] for r in res.results], axis=0).astype(np.float32)
    A = np.concatenate([r["A"] for r in res.results], axis=0).astype(np.float32)
    return Z.reshape(B, F), A.reshape(B, N, 1)

